# revision 1
# baseline (speedup 1.0000x reference)
"""Trainium2 Bass kernel for nn_AttnBlock (GroupNorm -> 1x1 q/k/v -> attention -> proj -> residual).

Input x: [4, 512, 64, 64] f32. Sharding: 8 cores = 4 batches x 2 query-halves.
Each core gets its batch's full x (columns permuted so its query half is first),
computes GroupNorm + full k/vT, q for its half, attention over all 4096 keys for
its 2048 queries, proj + residual, and returns [512, 2048].

Numerics: GroupNorm stats and softmax normalization in f32; all matmuls in
fp8e4m3 with DoubleRow packing (2x PE throughput), accumulating in f32 PSUM.
exp(s - 1) keeps attention weights inside the e4m3 normal range; attn@v stays
un-normalized (scaled 1/512 into fp8) and the 512/denominator factor is applied
after the output projection (division commutes with the channel mixing).

Layouts (per core):
  x_sb  [128, 4, 4096]    bf16  (stats + hn source; f32 x streamed for residual)
  k_f8  [128, 2, 2, 4096] fp8   c-pair-packed lhsT for scores^T
  q_f8  [128, 2, 2, 2048] fp8   c-pair-packed rhs for scores^T
  vT_f8 [128, 16, 2, 512] fp8   j-pair-packed lhsT for attn@v
Attention runs in scores^T[j, i] layout; the softmax denominator is a
DoubleRow ones-matmul (partition reduction on PE). fp8 attn tiles persist per
query chunk so attn@v runs ct-major in 2 PSUM banks, freeing banks to give
phase-1 and attention disjoint PSUM tags (phases overlap in the schedule).
"""

import numpy as np
import ml_dtypes

import concourse.bass as bass
import concourse.mybir as mybir
import concourse.tile as tile
from concourse.vector_clock import ScopedClock
from concourse.bass_utils import run_bass_kernel_spmd

F32 = mybir.dt.float32
F32R = mybir.dt.float32r
BF16 = mybir.dt.bfloat16
FP8 = mybir.dt.float8e4
AF = mybir.ActivationFunctionType
ALU = mybir.AluOpType

P = 128
C = 512          # channels
N = 4096         # spatial positions (64*64)
NQ = 2048        # queries per core (half)
CT = C // P      # 4 channel tiles
JC = N // 512    # 8 key chunks of 512
JT = N // P      # 32 key tiles of 128
ICH = NQ // 512  # 4 query chunks of 512
NUM_GROUPS = 16
GSIZE = C // NUM_GROUPS            # 32 channels per group
G_ELEMS = GSIZE * N                # elements per group
EPS = 1e-6
SCALE = float(C) ** -0.5


class PatchedTileContext(tile.TileContext):
    """walrus in this container accepts only ONE sync-wait per instruction;
    split extra waits onto same-engine NoOps placed just before the
    instruction (same queue => waits still execute before it)."""

    def _lower_ordered_insts(self, ordered):
        for bb_name, insts in list(ordered.items()):
            new_list = []
            for inst in insts:
                si = inst.sync_info
                if si is not None and si.on_wait and len(si.on_wait) > 1:
                    waits = list(si.on_wait)
                    for w in waits[:-1]:
                        nop = mybir.InstNoOp(
                            name=self.nc.get_next_instruction_name(),
                            engine=inst.engine,
                            sync_info=mybir.SyncInfo(on_wait=[w], on_update=[]),
                            bass_nofuse=True,
                        )
                        new_list.append(nop)
                    si.on_wait = [waits[-1]]
                new_list.append(inst)
            ordered[bb_name] = new_list
        super()._lower_ordered_insts(ordered)

    def _drain_and_barrier(self, tick_clock, wait_clock):
        drain_inst = self.nc.sync.drain()
        wait_clock.add_sem_waits(
            drain_inst.ins, ScopedClock({None: tick_clock.global_clock})
        )
        si = drain_inst.ins.sync_info
        if si is not None and si.on_wait and len(si.on_wait) > 1:
            waits = list(si.on_wait)
            si.on_wait = [waits[0]]
            for w in waits[1:]:
                d2 = self.nc.sync.drain()
                d2.ins.sync_info = mybir.SyncInfo(on_wait=[w], on_update=[])
        self.nc.all_engine_barrier()
        assert self.sems is not None
        popped = self.nc._tile_sem_poison_stack.pop()
        assert popped is self._sem_poison
        self.nc.clear_and_free_semaphores(list(self.sems.allocated().values()))
        self.nc.all_engine_barrier()


def build_nc(reps=1):
    nc = bass.Bass(name=f"attnblk_r{reps}")

    x_d = nc.dram_tensor("x", [C, N], F32, kind="ExternalInput")
    xbf_d = nc.dram_tensor("xbf", [C, N], BF16, kind="ExternalInput")
    wqtf8_d = nc.dram_tensor("wqtf8", [P, 4 * 512], FP8, kind="ExternalInput")
    wktf8_d = nc.dram_tensor("wktf8", [P, 4 * 512], FP8, kind="ExternalInput")
    wvtf8_d = nc.dram_tensor("wvtf8", [P, 4 * 512], FP8, kind="ExternalInput")
    wptf8_d = nc.dram_tensor("wptf8", [P, 4 * 512], FP8, kind="ExternalInput")
    gamma_d = nc.dram_tensor("gamma", [C], F32, kind="ExternalInput")
    beta_d = nc.dram_tensor("beta", [C], F32, kind="ExternalInput")
    bq_d = nc.dram_tensor("bq", [C], F32, kind="ExternalInput")
    bk_d = nc.dram_tensor("bk", [C], F32, kind="ExternalInput")
    bv_d = nc.dram_tensor("bv", [C], F32, kind="ExternalInput")
    bp_d = nc.dram_tensor("bp", [C], F32, kind="ExternalInput")
    g4_d = nc.dram_tensor("g4", [P, 4], F32, kind="ExternalInput")
    g4t_d = nc.dram_tensor("g4t", [4, P], F32, kind="ExternalInput")
    onesr_d = nc.dram_tensor("onesr", [1, P], F32R, kind="ExternalInput")
    out_d = nc.dram_tensor("out", [C, NQ], F32, kind="ExternalOutput")

    with PatchedTileContext(nc) as tc:
        with (
            tc.tile_pool(name="const", bufs=1) as const,
            tc.tile_pool(name="persist", bufs=1) as persist,
            tc.tile_pool(name="small", bufs=4) as small,
            tc.tile_pool(name="hnp", bufs=3) as hnp,
            tc.tile_pool(name="atp", bufs=34) as atp,
            tc.tile_pool(name="o2np", bufs=2) as o2np,
            tc.tile_pool(name="finp", bufs=3) as finp,
            tc.tile_pool(name="ps", bufs=1, space="PSUM") as ps,
        ):
            # ---------------- constants ----------------
            wqt_f8 = const.tile([P, 2, 2, C], FP8)
            nc.gpsimd.dma_start(wqt_f8[:], wqtf8_d[:, :].rearrange("p (kp s co) -> p kp s co", kp=2, s=2))
            wkt_f8 = const.tile([P, 2, 2, C], FP8)
            nc.gpsimd.dma_start(wkt_f8[:], wktf8_d[:, :].rearrange("p (kp s co) -> p kp s co", kp=2, s=2))
            wvt_f8 = const.tile([P, 2, 2, C], FP8)
            nc.gpsimd.dma_start(wvt_f8[:], wvtf8_d[:, :].rearrange("p (kp s co) -> p kp s co", kp=2, s=2))
            wpt_f8 = const.tile([P, 2, 2, C], FP8)
            nc.gpsimd.dma_start(wpt_f8[:], wptf8_d[:, :].rearrange("p (kp s co) -> p kp s co", kp=2, s=2))

            gam = const.tile([P, CT], F32)
            nc.gpsimd.dma_start(gam[:], gamma_d[:].rearrange("(t p) -> p t", p=P))
            bet = const.tile([P, CT], F32)
            nc.gpsimd.dma_start(bet[:], beta_d[:].rearrange("(t p) -> p t", p=P))
            bq4 = const.tile([P, CT], F32)
            nc.gpsimd.dma_start(bq4[:], bq_d[:].rearrange("(t p) -> p t", p=P))
            bk4 = const.tile([P, CT], F32)
            nc.gpsimd.dma_start(bk4[:], bk_d[:].rearrange("(t p) -> p t", p=P))
            bp4 = const.tile([P, CT], F32)
            nc.gpsimd.dma_start(bp4[:], bp_d[:].rearrange("(t p) -> p t", p=P))
            g4_sb = const.tile([P, 4], F32)
            nc.gpsimd.dma_start(g4_sb[:], g4_d[:, :])
            g4t_sb = const.tile([4, P], F32)
            nc.gpsimd.dma_start(g4t_sb[:], g4t_d[:, :])

            ones_row = const.tile([1, P], F32R)
            nc.gpsimd.dma_start(ones_row[:], onesr_d[:, :])
            eps_sb = const.tile([P, 1], F32)
            nc.vector.memset(eps_sb[:], EPS)
            bias_m1 = const.tile([P, 1], F32)
            nc.vector.memset(bias_m1[:], -1.0)
            ones_f8 = const.tile([P, 2, 16], FP8)
            nc.vector.memset(ones_f8[:], 1.0)

            # bv broadcast [128, 512] (v bias lives on the free dim of vT)
            bvb = persist.tile([P, C], F32)
            nc.sync.dma_start(
                bvb[:], bv_d[:].rearrange("(a c) -> a c", a=1).to_broadcast([P, C])
            )

            # ---------------- x resident (bf16: stats + hn inputs) ----------------
            x_sb = persist.tile([P, CT, N], BF16)

            k_f8 = persist.tile([P, CT // 2, 2, N], FP8)
            vT_f8 = persist.tile([P, JT // 2, 2, 512], FP8)
            q_f8 = persist.tile([P, CT // 2, 2, NQ], FP8)
            scale_sb = persist.tile([P, CT], F32)
            bias_sb = persist.tile([P, CT], F32)

            for _rep in range(reps):
              for ct in range(CT):
                  for xh in range(2):
                      eng = nc.sync if (2 * ct + xh) % 2 == 0 else nc.scalar
                      eng.dma_start(
                          x_sb[:, ct, xh * 2048:(xh + 1) * 2048],
                          xbf_d[ct * P:(ct + 1) * P, xh * 2048:(xh + 1) * 2048],
                      )
              if True:
                  # ---------------- phase 0: groupnorm stats ----------------
                  # red per ct = (mean_c, E_c[x^2]) [P, 2]; g4 is host-scaled by
                  # 1/GSIZE so the group matmul directly yields (mu_g, E_g[x^2]).
                  # ct0 computed on ACT (sum + sumsq accum), ct1-3 on DVE (bn_stats).
                  mrall = small.tile([4, 8], F32, tag="mrall")
                  for ct in range(CT):
                      red = small.tile([P, 2], F32, tag="red", name=f"red_{ct}")
                      if ct == 0:
                          # ACT route: sums/sumsq accumulate while the x DMA streams
                          reds = small.tile([P, JC], F32, tag="reds")
                          redq = small.tile([P, JC], F32, tag="redq")
                          for jc in range(JC):
                              cp = hnp.tile([P, 512], F32, tag="cp", name=f"cp_{jc}")
                              nc.scalar.activation(
                                  cp[:], x_sb[:, ct, jc * 512:(jc + 1) * 512], AF.Copy,
                                  accum_out=reds[:, jc:jc + 1],
                              )
                              sq = hnp.tile([P, 512], F32, tag="sq", name=f"sq_{jc}")
                              nc.scalar.activation(
                                  sq[:], x_sb[:, ct, jc * 512:(jc + 1) * 512], AF.Square,
                                  accum_out=redq[:, jc:jc + 1],
                              )
                          rsum = small.tile([P, 2], F32, tag="rsum")
                          nc.vector.reduce_sum(rsum[:, 0:1], reds[:], axis=mybir.AxisListType.X)
                          nc.vector.reduce_sum(rsum[:, 1:2], redq[:], axis=mybir.AxisListType.X)
                          nc.vector.tensor_scalar_mul(red[:], rsum[:], 1.0 / N)
                      else:
                          bnst = small.tile([P, JC, 6], F32, tag="bnst", name=f"bnst_{ct}")
                          for jc in range(JC):
                              nc.vector.bn_stats(bnst[:, jc, :], x_sb[:, ct, jc * 512:(jc + 1) * 512])
                          mv = small.tile([P, 2], F32, tag="mv", name=f"mv_{ct}")
                          nc.vector.bn_aggr(mv[:], bnst[:])
                          msq = small.tile([P, 1], F32, tag="msq", name=f"msq_{ct}")
                          nc.scalar.activation(msq[:], mv[:, 0:1], AF.Square)
                          nc.scalar.copy(red[:, 0:1], mv[:, 0:1])
                          nc.vector.tensor_tensor(red[:, 1:2], mv[:, 1:2], msq[:], ALU.add)
                      gps = ps.tile([4, 2], F32, tag="den", bufs=1, name=f"gps_{ct}")
                      nc.tensor.matmul(
                          gps[:], lhsT=g4_sb[:], rhs=red[:],
                          start=True, stop=True,
                      )
                      nc.scalar.copy(mrall[:, ct:ct + 1], gps[:, 0:1])
                      nc.scalar.copy(mrall[:, 4 + ct:5 + ct], gps[:, 1:2])
                  # mu = mrall[:, :4]; var = mrall[:, 4:] - mu^2 (batched)
                  musq = small.tile([4, 4], F32, tag="musq")
                  nc.scalar.activation(musq[:], mrall[:, 0:4], AF.Square)
                  var4 = small.tile([4, 4], F32, tag="var4")
                  nc.vector.tensor_tensor(var4[:], mrall[:, 4:8], musq[:], ALU.subtract)
                  std4 = small.tile([4, 4], F32, tag="std4")
                  nc.scalar.activation(std4[:], var4[:], AF.Sqrt, bias=eps_sb[0:4, :])
                  nc.vector.reciprocal(mrall[:, 4:8], std4[:])
                  # one bcast matmul: [128, 8] = (mu | rstd) per channel
                  mrp = ps.tile([P, 8], F32, tag="den", bufs=1, name="mrp")
                  nc.tensor.matmul(
                      mrp[:], lhsT=g4t_sb[:], rhs=mrall[:],
                      start=True, stop=True,
                  )
                  # scale = gamma * rstd ; bias = beta - mu * scale (batched)
                  nc.vector.tensor_tensor(scale_sb[:], gam[:], mrp[:, 4:8], ALU.mult)
                  tb = small.tile([P, 4], F32, tag="tb")
                  nc.vector.tensor_tensor(tb[:], mrp[:, 0:4], scale_sb[:], ALU.mult)
                  nc.vector.tensor_tensor(bias_sb[:], bet[:], tb[:], ALU.subtract)

                  # ---------------- phase 1: hn -> k, vT, q ----------------
                  for jc in range(JC):
                      hn8 = hnp.tile([P, 2, 2, 512], FP8, tag="hn8")
                      for kc in range(CT):
                          nc.vector.tensor_scalar(
                              hn8[:, kc // 2, kc % 2, :], x_sb[:, kc, jc * 512:(jc + 1) * 512],
                              scale_sb[:, kc:kc + 1], bias_sb[:, kc:kc + 1],
                              ALU.mult, ALU.add,
                          )
                      for co in range(CT):
                          pk = ps.tile([P, 512], F32, tag="pp", bufs=2, name="pk")
                          for kp in range(2):
                              nc.tensor.matmul(
                                  pk[:], lhsT=wkt_f8[:, kp, :, co * P:(co + 1) * P], rhs=hn8[:, kp],
                                  perf_mode=mybir.MatmulPerfMode.DoubleRow,
                                  start=(kp == 0), stop=(kp == 1),
                              )
                          nc.scalar.activation(
                              k_f8[:, co // 2, co % 2, jc * 512:(jc + 1) * 512], pk[:],
                              AF.Identity, bias=bk4[:, co:co + 1],
                          )
                      for jl in range(4):
                          jt = jc * 4 + jl
                          pv = ps.tile([P, 512], F32, tag="pp", bufs=2, name="pv")
                          for kp in range(2):
                              nc.tensor.matmul(
                                  pv[:], lhsT=hn8[:, kp, :, jl * P:(jl + 1) * P], rhs=wvt_f8[:, kp],
                                  perf_mode=mybir.MatmulPerfMode.DoubleRow,
                                  start=(kp == 0), stop=(kp == 1),
                              )
                          nc.vector.tensor_tensor(vT_f8[:, jt // 2, jt % 2, :], pv[:], bvb[:], ALU.add)
                      if jc < ICH:
                          for co in range(CT):
                              pq = ps.tile([P, 512], F32, tag="pp", bufs=2, name="pq")
                              for kp in range(2):
                                  nc.tensor.matmul(
                                      pq[:], lhsT=wqt_f8[:, kp, :, co * P:(co + 1) * P], rhs=hn8[:, kp],
                                      perf_mode=mybir.MatmulPerfMode.DoubleRow,
                                      start=(kp == 0), stop=(kp == 1),
                                  )
                              nc.vector.tensor_scalar(
                                  q_f8[:, co // 2, co % 2, jc * 512:(jc + 1) * 512], pq[:],
                                  bq4[:, co:co + 1], None, ALU.add,
                              )

              # ---------------- phase 2: attention ----------------
              if True:
                  for ich in range(ICH):
                      den = ps.tile([1, 512], F32, tag="den", bufs=1, name=f"den_{ich}")
                      # residual + out-proj bias staged early, off the critical path
                      xqb = o2np.tile([P, CT, 512], F32, tag="xqb")
                      for ot in range(CT):
                          xq = finp.tile([P, 512], F32, tag="xq", name=f"xq_{ich}_{ot}")
                          nc.sync.dma_start(
                              xq[:], x_d[ot * P:(ot + 1) * P, ich * 512:(ich + 1) * 512]
                          )
                          nc.gpsimd.tensor_scalar(
                              xqb[:, ot, :], xq[:], bp4[:, ot:ot + 1], None, ALU.add
                          )
                      at2s = []
                      for t in range(JT // 2):
                          at2 = atp.tile([P, 2, 512], FP8, tag="at", name=f"at2_{ich}_{t}")
                          at2s.append(at2)
                          for s in range(2):
                              jt = 2 * t + s
                              pssc = ps.tile([P, 512], F32, tag="sc", bufs=3, name="pssc")
                              for kp in range(CT // 2):
                                  nc.tensor.matmul(
                                      pssc[:], lhsT=k_f8[:, kp, :, jt * P:(jt + 1) * P],
                                      rhs=q_f8[:, kp, :, ich * 512:(ich + 1) * 512],
                                      perf_mode=mybir.MatmulPerfMode.DoubleRow,
                                      start=(kp == 0), stop=(kp == CT // 2 - 1),
                                  )
                              nc.scalar.activation(at2[:, s, :], pssc[:], AF.Exp, scale=SCALE, bias=bias_m1[:])
                          nc.tensor.matmul(
                              den[:], lhsT=ones_f8[:, :, 0:1], rhs=at2[:],
                              perf_mode=mybir.MatmulPerfMode.DoubleRow,
                              start=(t == 0), stop=(t == JT // 2 - 1),
                          )
                      # attn@v over the persisted fp8 tiles, un-normalized
                      # (1/denominator applied after proj). ct-major uses one o2
                      # bank at a time; the final chunk goes tile-major across 4
                      # banks (2 borrowed from the retired phase-1 tag) so its
                      # tail is not serialized behind the full j-loop.
                      o2n = o2np.tile([P, 2, 2, 512], FP8, tag="o2n")
                      if ich < ICH - 1:
                          for ct in range(CT):
                              o2t = ps.tile([P, 512], F32, tag="o2", bufs=2, name=f"o2_{ich}_{ct}")
                              for t in range(JT // 2):
                                  nc.tensor.matmul(
                                      o2t[:], lhsT=vT_f8[:, t, :, ct * P:(ct + 1) * P], rhs=at2s[t][:],
                                      perf_mode=mybir.MatmulPerfMode.DoubleRow,
                                      start=(t == 0), stop=(t == JT // 2 - 1),
                                  )
                              nc.vector.tensor_scalar(
                                  o2n[:, ct // 2, ct % 2, :], o2t[:], 1.0 / 512.0, None, ALU.mult
                              )
                      else:
                          o2l = [
                              ps.tile([P, 512], F32, tag=("o2" if i < 2 else "pp"),
                                      bufs=2, name=f"o2l_{i}")
                              for i in range(CT)
                          ]
                          for t in range(JT // 2):
                              for ct in range(CT):
                                  nc.tensor.matmul(
                                      o2l[ct][:], lhsT=vT_f8[:, t, :, ct * P:(ct + 1) * P],
                                      rhs=at2s[t][:],
                                      perf_mode=mybir.MatmulPerfMode.DoubleRow,
                                      start=(t == 0), stop=(t == JT // 2 - 1),
                                  )
                          for ct in range(CT):
                              if ct % 2 == 0:
                                  nc.scalar.mul(o2n[:, ct // 2, ct % 2, :], o2l[ct][:], 1.0 / 512.0)
                              else:
                                  nc.vector.tensor_scalar(
                                      o2n[:, ct // 2, ct % 2, :], o2l[ct][:], 1.0 / 512.0, None, ALU.mult
                                  )
                      rec = small.tile([1, 512], F32R, tag="rec")
                      with nc.allow_low_precision(reason="f32r softmax denom reciprocal"):
                          nc.vector.reciprocal(rec[:], den[:])
                      rbp = ps.tile([P, 512], F32, tag="pp", bufs=2, name=f"rbp_{ich}")
                      nc.tensor.matmul(rbp[:], lhsT=ones_row[:], rhs=rec[:], start=True, stop=True)
                      rb = finp.tile([P, 512], F32, tag="rb")
                      nc.vector.tensor_copy(rb[:], rbp[:])
                      for ot in range(CT):
                          p3 = ps.tile([P, 512], F32, tag="pp", bufs=2, name="p3")
                          for kp in range(2):
                              nc.tensor.matmul(
                                  p3[:], lhsT=wpt_f8[:, kp, :, ot * P:(ot + 1) * P], rhs=o2n[:, kp],
                                  perf_mode=mybir.MatmulPerfMode.DoubleRow,
                                  start=(kp == 0), stop=(kp == 1),
                              )
                          fin = finp.tile([P, 512], F32, tag="fin")
                          nc.vector.tensor_tensor(fin[:], p3[:], rb[:], ALU.mult)
                          nc.vector.tensor_tensor(fin[:], fin[:], xqb[:, ot, :], ALU.add)
                          nc.sync.dma_start(
                              out_d[ot * P:(ot + 1) * P, ich * 512:(ich + 1) * 512], fin[:]
                          )
    return nc


_NC = None


def _get_nc():
    global _NC
    if _NC is None:
        _NC = build_nc()
    return _NC


def _make_in_maps(x, gamma, beta, wq, bq, wk, bk, wv, bv, wp, bp):
    x = np.ascontiguousarray(np.asarray(x, dtype=np.float32)).reshape(4, C, N)
    bf = ml_dtypes.bfloat16
    def pack8(w):
        return np.ascontiguousarray(
            np.asarray(w, np.float32).T.reshape(2, 2, P, 512).transpose(2, 0, 1, 3)
            .reshape(P, 4 * 512).astype(mybir.dt.np(FP8))
        )


    g4i = np.zeros((P, 4), np.float32)
    for p in range(P):
        g4i[p, p // GSIZE] = 1.0
    g4 = g4i / GSIZE          # group-mean matmul (pre-scaled)
    g4t = np.ascontiguousarray(g4i.T)  # broadcast indicator (0/1)
    common = {
        "wqtf8": pack8(wq), "wktf8": pack8(wk),
        "wptf8": np.ascontiguousarray(
            np.asarray(wp, np.float32).T.reshape(2, 2, P, 512).transpose(2, 0, 1, 3)
            .reshape(P, 4 * 512).astype(mybir.dt.np(FP8))
        ),
        "wvtf8": np.ascontiguousarray(
            np.asarray(wv, np.float32).T.reshape(2, 2, P, 512).transpose(2, 0, 1, 3)
            .reshape(P, 4 * 512).astype(mybir.dt.np(FP8))
        ),
        "gamma": np.asarray(gamma, np.float32), "beta": np.asarray(beta, np.float32),
        "bq": np.asarray(bq, np.float32), "bk": np.asarray(bk, np.float32),
        "bv": np.asarray(bv, np.float32), "bp": np.asarray(bp, np.float32),
        "g4": g4, "g4t": g4t,
        "onesr": np.full((1, P), 512.0, np.float32),
    }
    in_maps = []
    for core in range(8):
        bidx, half = core // 2, core % 2
        xb = x[bidx]
        if half == 0:
            xp = xb
        else:
            xp = np.concatenate([xb[:, NQ:], xb[:, :NQ]], axis=1)
        xp = np.ascontiguousarray(xp)
        in_maps.append({"x": xp, "xbf": xp.astype(bf), **common})
    return in_maps


def run(inputs, trace=False):
    nc = _get_nc()
    in_maps = _make_in_maps(**inputs)
    res = run_bass_kernel_spmd(nc, in_maps, list(range(8)), trace=trace)
    out = np.empty((4, C, N), np.float32)
    for core in range(8):
        bidx, half = core // 2, core % 2
        o = res.results[core]["out"]
        if half == 0:
            out[bidx, :, :NQ] = o
        else:
            out[bidx, :, NQ:] = o
    return out.reshape(4, C, 64, 64), res


def kernel(**inputs):
    out, _ = run(inputs, trace=False)
    return out

